# revision 8
# baseline (speedup 1.0000x reference)
"""Trainium2 Bass kernel for nn_AttentionDigitCaps (capsule dynamic routing).

reference math:
    x = inputs.reshape(B, N, iL)                      # B=32, N=2048, iL=32
    u = einsum('bji,jik->bjk', x, W).reshape(B,N,C,L) # C=L=32
    b = 0; for r in 3: c = softmax(b, C); s = sum_j u*c + biases; v = squash(s)
                       if r<2: b += sum_l u*v

ONE launch, capsule-sharded (256 j per core), u never leaves SBUF:

phase 1: u = x @ W in bf16 (W streamed 16.8 MB/core, PE at full bf16 rate,
  psum evacuated alternately by ACT and DVE into a persistent SBUF tile),
  plus the s0 partial (sum_j u) via a tiled-eye(32) selector matmul.
AR0:     AllReduce the [B, CL] f32 s0 partial across the 8 cores (128 KB via
  DRAM bounce buffers), then on-core: s0/C + bias -> squash -> v1,
  replicated to 128 partitions with a K=32 matmul.
iter r=1: per 8-column chunk: b += sum_l u*v (bf16 mul + in-place add-tree
  over l), c = softmax_c(b) (ACT exp + DVE), s1 += sel.T @ (u*c) on the PE.
AR1:     AllReduce s1, + bias, squash, replicate -> v2.
iter r=2: same, producing the s2 partial, which goes back to the host
  (the final cross-core sum + squash happens there in f64).

Layout notes: partition p = (a in 4, b in 32) where a is 2 bits of the
local capsule index; free columns are (l, c) with c innermost, so the
l-tree adds and the c-softmax are contiguous and every big DVE op runs at
the bf16 tensor-tensor rate (~1.9 elem/ns/partition; TRN2 TT ops cannot
reach the 2-port 2x rate, which makes the two muls + tree the hard floor).
"""

import os
import sys
import numpy as np

if "/opt/trn_rl_repo" not in sys.path:
    sys.path.insert(0, "/opt/trn_rl_repo")

import ml_dtypes

BF16 = ml_dtypes.bfloat16

CORES = 8
B, N, IL, C, L = 32, 2048, 32, 32, 32
NLOC = N // CORES          # 256 capsules per core
G = NLOC // 16             # 16 W-stream groups of 16 capsules
M = NLOC // 4              # 64 capsule columns per core (4 capsules / matmul)
CL = C * L                 # 1024
EPS = 1e-7

_CACHE = {}


def _mk_nc():
    from concourse import bacc
    return bacc.Bacc("TRN2", target_bir_lowering=False, debug=False,
                     num_devices=CORES)


def _build_S():
    from concourse import tile
    import concourse.mybir as mybir

    f32 = mybir.dt.float32
    bf16 = mybir.dt.bfloat16
    AF = mybir.ActivationFunctionType
    OP = mybir.AluOpType
    AX = mybir.AxisListType

    nc = _mk_nc()
    xbd_p = nc.dram_tensor("xbd", [128, G, 4, 128], bf16, kind="ExternalInput")
    w_p = nc.dram_tensor("w", [G, 128, 4, CL], bf16, kind="ExternalInput")
    bones_p = nc.dram_tensor("bones", [128, B], bf16, kind="ExternalInput")
    repw_p = nc.dram_tensor("repw", [B, 128], bf16, kind="ExternalInput")
    bias_p = nc.dram_tensor("bias", [B, CL], f32, kind="ExternalInput")
    s2_out = nc.dram_tensor("s2", [B, CL], f32, kind="ExternalOutput")

    NK = 8          # chunk count per iteration
    KJ = M // NK    # capsule columns per chunk

    with tile.TileContext(nc) as tc:
        with (
            tc.tile_pool(name="persist", bufs=1) as pp,
            tc.tile_pool(name="dram", bufs=1, space="DRAM") as dramp,
        ):
            u_sb = pp.tile([128, M, CL], bf16)
            b_state = pp.tile([128, M, C], f32)
            bones = pp.tile([128, B], bf16)
            repw = pp.tile([B, 128], bf16)
            bias = pp.tile([B, CL], f32)
            nc.sync.dma_start(out=bones[:], in_=bones_p[:])
            nc.sync.dma_start(out=repw[:], in_=repw_p[:])
            nc.sync.dma_start(out=bias[:], in_=bias_p[:])
            ar_in = [dramp.tile([B, CL], f32, tag=f"ari{i}",
                                name=f"ar_in{i}") for i in (0, 1)]
            ar_out = [dramp.tile([B, CL], f32, tag=f"aro{i}",
                                 name=f"ar_out{i}") for i in (0, 1)]

            # ---------------- phase 1: u + s0 partial ----------------
            with (
                tc.tile_pool(name="xw", bufs=1) as xwp,
                tc.tile_pool(name="wst", bufs=2) as wsp,
                tc.tile_pool(name="p1ps", bufs=2, space="PSUM") as p1ps,
                tc.tile_pool(name="s0ps", bufs=1, space="PSUM") as s0psp,
            ):
                xbd = xwp.tile([128, G, 4, 128], bf16)
                nc.sync.dma_start(out=xbd[:], in_=xbd_p[:])
                s0_ps = s0psp.tile([B, CL], f32, tag="s0")
                for g in range(G):
                    w_t = wsp.tile([128, 4, CL], bf16, tag="w")
                    nc.sync.dma_start(out=w_t[:], in_=w_p[g])
                    for jc in range(4):
                        m = g * 4 + jc
                        ps = p1ps.tile([128, CL], f32, tag="ups")
                        for h in range(2):
                            nc.tensor.matmul(
                                ps[:, 512 * h:512 * h + 512],
                                xbd[:, g, jc, :],
                                w_t[:, jc, 512 * h:512 * h + 512],
                                start=True, stop=True)
                        if m % 2 == 0:
                            nc.scalar.activation(u_sb[:, m, :], ps[:], AF.Copy)
                        else:
                            nc.vector.tensor_copy(u_sb[:, m, :], ps[:])
                    for jc in range(4):
                        m = g * 4 + jc
                        for h in range(2):
                            nc.tensor.matmul(
                                s0_ps[:, 512 * h:512 * h + 512],
                                bones[:], u_sb[:, m, 512 * h:512 * h + 512],
                                start=(g == 0 and jc == 0),
                                stop=(g == G - 1 and jc == 3),
                                skip_group_check=True)
                s0_loc = pp.tile([B, CL], f32)
                nc.scalar.activation(s0_loc[:], s0_ps[:], AF.Copy)

            # ---------------- routing ----------------
            with (
                tc.tile_pool(name="work", bufs=1) as workp,
                tc.tile_pool(name="small", bufs=1) as smallp,
                tc.tile_pool(name="sps", bufs=2, space="PSUM") as spsp,
                tc.tile_pool(name="vps", bufs=1, space="PSUM") as vpsp,
            ):
                def allreduce_squash(i, part_sb, scale):
                    """AllReduce the [B, CL] partial, + bias, squash -> vrep."""
                    nc.gpsimd.dma_start(ar_in[i][:], part_sb[:])
                    nc.gpsimd.collective_compute(
                        "AllReduce", OP.add,
                        replica_groups=[list(range(CORES))],
                        ins=[ar_in[i].opt()], outs=[ar_out[i].opt()])
                    s_ar = smallp.tile([B, CL], f32, tag="sar")
                    nc.sync.dma_start(out=s_ar[:], in_=ar_out[i][:])
                    s_sb = smallp.tile([B, CL], f32, tag="ssb")
                    if scale != 1.0:
                        nc.vector.tensor_scalar(s_sb[:], s_ar[:], scale, None,
                                                op0=OP.mult)
                        nc.vector.tensor_add(s_sb[:], s_sb[:], bias[:])
                    else:
                        nc.vector.tensor_add(s_sb[:], s_ar[:], bias[:])
                    # squash: v = s * q/((1+q)(sqrt(q)+eps)), q = sum_l s^2
                    q2 = smallp.tile([B, CL], f32, tag="q2")
                    nc.vector.tensor_mul(q2[:], s_sb[:], s_sb[:])
                    q2v = q2.rearrange("p (l c) -> p l c", l=L)
                    for hw in (16, 8, 4, 2):
                        nc.vector.tensor_add(q2v[:, 0:hw, :], q2v[:, 0:hw, :],
                                             q2v[:, hw:2 * hw, :])
                    qs = smallp.tile([B, C], f32, tag="qs")
                    nc.vector.tensor_add(qs[:], q2v[:, 0, :], q2v[:, 1, :])
                    nrm = smallp.tile([B, C], f32, tag="nrm")
                    nc.scalar.activation(nrm[:], qs[:], AF.Sqrt)
                    ne = smallp.tile([B, C], f32, tag="ne")
                    nc.vector.tensor_scalar_add(ne[:], nrm[:], EPS)
                    q1 = smallp.tile([B, C], f32, tag="q1")
                    nc.vector.tensor_scalar_add(q1[:], qs[:], 1.0)
                    den = smallp.tile([B, C], f32, tag="den")
                    nc.vector.tensor_mul(den[:], ne[:], q1[:])
                    rden = smallp.tile([B, C], f32, tag="rden")
                    nc.vector.reciprocal(rden[:], den[:])
                    fac = smallp.tile([B, C], f32, tag="fac")
                    nc.vector.tensor_mul(fac[:], qs[:], rden[:])
                    vb16 = smallp.tile([B, CL], bf16, tag="vb16")
                    facb = fac.rearrange("p (x c) -> p x c", x=1)
                    facb = facb.broadcast_to([B, L, C])
                    nc.vector.tensor_mul(
                        vb16.rearrange("p (l c) -> p l c", l=L),
                        s_sb.rearrange("p (l c) -> p l c", l=L), facb)
                    vps = vpsp.tile([128, CL], f32, tag="vrep")
                    for h in range(2):
                        nc.tensor.matmul(
                            vps[:, 512 * h:512 * h + 512],
                            repw[:], vb16[:, 512 * h:512 * h + 512],
                            start=True, stop=True)
                    vrep = smallp.tile([128, CL], bf16, tag="vrep")
                    nc.scalar.activation(vrep[:], vps[:], AF.Copy)
                    return vrep

                vcur = allreduce_squash(0, s0_loc, 1.0 / C)

                for it in range(2):
                    s_ps = spsp.tile([B, CL], f32, tag="sps")

                    def flush(k, e):
                        z = workp.tile([128, KJ], f32, tag="z")
                        nc.vector.tensor_reduce(z[:], e[:], axis=AX.X,
                                                op=OP.add)
                        rz = workp.tile([128, KJ], f32, tag="rz")
                        nc.vector.reciprocal(rz[:], z[:])
                        cw = workp.tile([128, KJ, C], bf16, tag="cw")
                        rzb = rz.rearrange("p (j x) -> p j x", x=1)
                        rzb = rzb.broadcast_to([128, KJ, C])
                        nc.vector.tensor_mul(cw[:], e[:], rzb)
                        tmp = workp.tile([128, KJ, L, C], bf16, tag="tmp")
                        uc = u_sb[:, KJ * k:KJ * k + KJ, :]
                        uc = uc.rearrange("p j (l c) -> p j l c", l=L)
                        cwb = cw.rearrange("p j (x c) -> p j x c", x=1)
                        cwb = cwb.broadcast_to([128, KJ, L, C])
                        nc.vector.tensor_mul(tmp[:], uc, cwb)
                        for jj in range(KJ):
                            rhs = tmp[:, jj].rearrange("p l c -> p (l c)")
                            for h in range(2):
                                nc.tensor.matmul(
                                    s_ps[:, 512 * h:512 * h + 512],
                                    bones[:], rhs[:, 512 * h:512 * h + 512],
                                    start=(k == 0 and jj == 0),
                                    stop=(k == NK - 1 and jj == KJ - 1),
                                    skip_group_check=True)

                    carry = None
                    for k in range(NK):
                        uc = u_sb[:, KJ * k:KJ * k + KJ, :]
                        uc = uc.rearrange("p j (l c) -> p j l c", l=L)
                        t0 = workp.tile([128, KJ, L, C], bf16, tag="t0")
                        vb = vcur.rearrange("p (x l c) -> p x l c", x=1, l=L)
                        vb = vb.broadcast_to([128, KJ, L, C])
                        nc.vector.tensor_mul(t0[:], uc, vb)
                        for hw in (16, 8, 4, 2):
                            nc.vector.tensor_add(t0[:, :, 0:hw, :],
                                                 t0[:, :, 0:hw, :],
                                                 t0[:, :, hw:2 * hw, :])
                        bc = b_state[:, KJ * k:KJ * k + KJ, :]
                        if it == 0:
                            nc.vector.tensor_add(bc, t0[:, :, 0, :],
                                                 t0[:, :, 1, :])
                        else:
                            r5 = workp.tile([128, KJ, C], bf16, tag="r5")
                            nc.vector.tensor_add(r5[:], t0[:, :, 0, :],
                                                 t0[:, :, 1, :])
                            nc.vector.tensor_add(bc, bc, r5[:])
                        e = workp.tile([128, KJ, C], bf16, tag="e", bufs=2)
                        nc.scalar.activation(e[:], bc, AF.Exp)
                        if carry is not None:
                            flush(*carry)
                        carry = (k, e)
                    flush(*carry)

                    s_loc = smallp.tile([B, CL], f32, tag="sloc")
                    nc.scalar.activation(s_loc[:], s_ps[:], AF.Copy)
                    if it == 0:
                        vcur = allreduce_squash(1, s_loc, 1.0)
                    else:
                        nc.sync.dma_start(out=s2_out[:], in_=s_loc[:])

    nc.compile()
    return nc


def _host_prep(inputs, W):
    """Per-core bf16 inputs: block-diagonal x, W with (l,c) columns."""
    x = np.ascontiguousarray(inputs.reshape(B, N, IL), dtype=np.float32)
    # x_sh[r, (a,i), g, jc, b] = x[b, r*256+g*16+a*4+jc, i]
    xr = x.reshape(B, CORES, G, 4, 4, IL)
    x_sh = xr.transpose(1, 3, 5, 2, 4, 0).reshape(CORES, 128, G, 4, B)
    xbd = np.zeros((CORES, 128, G, 4, 128), np.float32)
    for a in range(4):
        xbd[:, 32 * a:32 * a + 32, :, :, 32 * a:32 * a + 32] = \
            x_sh[:, 32 * a:32 * a + 32]
    xbd = np.ascontiguousarray(xbd).astype(BF16)
    # w_sh[r, g, (a,i), jc, (l,c)] = W[r*256+g*16+a*4+jc, i, c*L+l]
    wr = np.asarray(W, np.float32).reshape(CORES, G, 4, 4, IL, C, L)
    w_sh = np.ascontiguousarray(
        wr.transpose(0, 1, 2, 4, 3, 6, 5).reshape(CORES, G, 128, 4, CL)
    ).astype(BF16)
    bones = np.ascontiguousarray(
        np.tile(np.eye(B, dtype=np.float32), (4, 1))).astype(BF16)
    repw = np.ascontiguousarray(
        np.tile(np.eye(B, dtype=np.float32), (1, 4))).astype(BF16)
    return xbd, w_sh, bones, repw


def _squash_np(s):
    """reference squash in float64; s is [B, C, L]."""
    s = s.astype(np.float64)
    n = np.linalg.norm(s, axis=-1, keepdims=True)
    return (n ** 2 / (1 + n ** 2) / (n + EPS)) * s


def _install_trace_hook():
    """Register the NTFF profiling hook (antenv.axon_hooks is absent in this
    container, but the ctypes implementation ships in trn_agent_boot)."""
    import types

    if "antenv.axon_hooks" in sys.modules:
        return
    try:
        from trn_agent_boot.trn_boot import _ntff_profile_via_ctypes
        hook = _ntff_profile_via_ctypes("/opt/axon/libaxon_pjrt.so")
        if hook is None:
            return
        m = types.ModuleType("antenv.axon_hooks")
        m.get_axon_ntff_profile_hook = lambda: hook
        sys.modules["antenv.axon_hooks"] = m
        from concourse import bass_utils
        bass_utils.upload_artifacts = lambda tmpdir: tmpdir  # no egress
    except Exception as e:  # profiling is best-effort
        print(f"trace hook install failed: {e}", file=sys.stderr)


def kernel(inputs, W, biases):
    from concourse.bass_utils import run_bass_kernel_spmd

    if "gs" not in _CACHE:
        _CACHE["gs"] = _build_S()
    gs = _CACHE["gs"]

    xbd, w_sh, bones, repw = _host_prep(inputs, W)
    biasT = np.asarray(biases, np.float32).T.reshape(1, CL)  # (l,c) order
    bias32 = np.ascontiguousarray(np.tile(biasT, (B, 1)))
    biases64 = np.asarray(biases, dtype=np.float64)

    trace = os.environ.get("KERNEL_TRACE", "0") == "1"
    if trace:
        _install_trace_hook()
    cores = list(range(CORES))

    res = run_bass_kernel_spmd(
        gs,
        [{"xbd": xbd[r], "w": w_sh[r], "bones": bones, "repw": repw,
          "bias": bias32} for r in cores],
        core_ids=cores, trace=trace)
    _CACHE["last_results"] = [res]

    s2 = sum(np.asarray(res.results[r]["s2"], np.float64) for r in cores)
    s2 = s2.reshape(B, L, C).transpose(0, 2, 1) + biases64
    v = _squash_np(s2)
    return np.ascontiguousarray(v.astype(np.float32))
